# revision 9
# baseline (speedup 1.0000x reference)
"""CenterLoss kernel for Trainium2 (8 NeuronCores, data-parallel over batch).

loss = mean_i( ||nx_i||^2 + ||c_{l_i}||^2 - 2*nx_i.c_{l_i} )
     = mean_i( ||nx_i - c_{l_i}||^2 ),  nx_i = x_i / max(||x_i||, EPS)

The reference's (batch, num_classes) distmat is masked down to one column
per row, so only a gather of centers[labels] is needed (memory regime).

Sharding: batch 16384 -> 8 cores x 2048 rows, centers replicated. x row
p*16+j of a core's shard lives at SBUF partition p, free block j
(natural contiguous DMA). The centers gather uses InstDMAGatherAnt
(gpsimd dma_gather), which is Q7 descriptor-generation bound (~8.5ns
per gathered row on one tx/rx core pair; hot-labels probe showed no
DRAM-locality sensitivity). The ucode runs queue q's desc-gen on Q7
core pair q, so the gather is split into NGATH gathers on distinct
SWDGE queues to parallelize desc-gen across core pairs. Gather slot i
writes dst[i % 128, i // 128]; labels are permuted host-side (slot
j*128+p = label of x row p*16+j) and pre-wrapped into the int16
[16, num_idxs/16] layout (replicated to all 8 Q7 core groups). Each
gather gets its own semaphore (a shared sem races: G>=16 could be a
mix of both gathers' per-engine completions). The mlp Q7 library is
loaded at program start so its ~6us IRAM load overlaps the input DMAs;
a dummy sqrt preloads the ACT table likewise. The x-side pipeline
(square/rowsum/sqrt/recip/normalize) runs under the gathers on
ACT+DVE; d = nx - c and the Square+accumulate reduction are chunked
per gather. Raw bacc with manual semaphores. Each core returns
per-partition partial sums; the host combines.
"""

import numpy as np

B, C, D = 16384, 8192, 64
N_CORES = 8
ROWS = B // N_CORES        # 2048
P = 128
J = ROWS // P              # 16 blocks of D per partition
F = J * D                  # 1024 f32 per partition
NGATH = 4                  # dma_gather instructions, one per SWDGE queue
GROWS = ROWS // NGATH      # idxs per gather
GBLK = J // NGATH          # J-blocks per gather
CPG = 1                    # compute chunks per gather
NCH = NGATH * CPG          # total compute chunks
CBLK = J // NCH            # J-blocks per compute chunk

_CACHE = {}


def _build():
    from contextlib import ExitStack

    import concourse.bass as bass
    from concourse import bacc, library_config, mybir

    nc = bacc.Bacc("TRN2", target_bir_lowering=False, debug=False,
                   num_devices=N_CORES, dynamic_dma_scratch_size=65536,
                   num_swdge_queues=NGATH)
    f32 = mybir.dt.float32
    x = nc.dram_tensor("x", [ROWS, D], f32, kind="ExternalInput").ap()
    labels = nc.dram_tensor("labels", [P, ROWS // 16], mybir.dt.int16,
                            kind="ExternalInput").ap()
    centers = nc.dram_tensor("centers", [C, D], f32,
                             kind="ExternalInput").ap()
    out = nc.dram_tensor("out", [P, NCH], f32, kind="ExternalOutput").ap()

    with ExitStack() as ctx:
        def sb(n, s, dt=f32):
            return ctx.enter_context(nc.sbuf_tensor(n, s, dt))
        lab_t = sb("lab_t", [P, ROWS // 16], mybir.dt.int16)
        x_t = sb("x_t", [P, F])
        c_t = sb("c_t", [P, F])
        xx = sb("xx", [P, F])
        sx = sb("sx", [P, J])
        mn = sb("mn", [P, J])
        inv = sb("inv", [P, J])
        nx = sb("nx", [P, F])
        acc = sb("acc", [P, NCH])
        L = ctx.enter_context(nc.semaphore("Lsem"))
        X = ctx.enter_context(nc.semaphore("Xsem"))
        G = [ctx.enter_context(nc.semaphore(f"Gsem{g}")) for g in range(NGATH)]
        A = ctx.enter_context(nc.semaphore("Asem"))   # ACT-produced events
        V = ctx.enter_context(nc.semaphore("Vsem"))   # DVE-produced events

        # ---- Sync: labels in, result out ----
        nc.sync.dma_start(lab_t[:], labels[:]).then_inc(L, 16)
        nc.sync.wait_ge(A, 2 + NCH)
        nc.sync.dma_start(out, acc[:]).then_inc(L, 16)
        nc.sync.wait_ge(L, 32)

        # ---- GpSimd: the centers gather ----
        # Load the mlp library first so its IRAM load overlaps input DMAs.
        nc.gpsimd.load_library(library_config.mlp)
        nc.gpsimd.wait_ge(L, 16)
        # gather g covers slots [g*GROWS, (g+1)*GROWS) on SWDGE queue g
        # (queue q's desc-gen runs on Q7 core pair q -> parallel).
        for g in range(NGATH):
            nc.gpsimd.dma_gather(
                c_t[:, g * GBLK * D:(g + 1) * GBLK * D].rearrange(
                    "p (j d) -> p j d", d=D),
                centers[:],
                lab_t[:, g * (GROWS // 16):(g + 1) * (GROWS // 16)],
                GROWS, GROWS, D, queue_num=g,
            ).then_inc(G[g], 16)

        # ---- Scalar/ACT: x in on its HWDGE ring, squares ----
        # A events: 1=xx, 2=mn(sqrt), 2+k+1 = chunk k accumulated
        nc.scalar.dma_start(x_t[:], x.rearrange("(p j) d -> p (j d)", p=P)
                            ).then_inc(X, 16)
        # Dummy sqrt (scale=0, bias=1 -> sqrt(1)) pulls the ACT table load
        # under the DMA shadow; mn is rewritten by the real sqrt below.
        nc.scalar.activation(mn[:, :1], mn[:, :1],
                             mybir.ActivationFunctionType.Sqrt,
                             bias=1.0, scale=0.0)
        nc.scalar.wait_ge(X, 16)
        nc.scalar.square(xx[:], x_t[:]).then_inc(A, 1)
        nc.scalar.wait_ge(V, 1)
        nc.scalar.sqrt(mn[:], sx[:]).then_inc(A, 1)
        for k in range(NCH):
            f0 = k * CBLK * D
            nc.scalar.wait_ge(V, 4 + k)
            nc.scalar.activation(c_t[:, f0:f0 + CBLK * D],
                                 c_t[:, f0:f0 + CBLK * D],
                                 mybir.ActivationFunctionType.Square,
                                 accum_out=acc[:, k:k + 1]).then_inc(A, 1)

        # ---- Vector/DVE ----
        # V events: 1=sx, 2=inv, 3=nx, 3+k+1 = chunk k sub done
        nc.vector.wait_ge(A, 1)
        nc.vector.reduce_sum(sx[:], xx[:].rearrange("p (j d) -> p j d", d=D),
                             axis=mybir.AxisListType.X).then_inc(V, 1)
        nc.vector.wait_ge(A, 2)
        nc.vector.reciprocal(inv[:], mn[:]).then_inc(V, 1)
        nc.vector.wait_ge(V, 2)
        iap = inv[:]
        inv_bc = bass.AP(tensor=iap.tensor, offset=iap.offset,
                         ap=list(iap.ap) + [[0, D]])
        nc.vector.tensor_tensor(
            out=nx[:].rearrange("p (j d) -> p j d", d=D),
            in0=x_t[:].rearrange("p (j d) -> p j d", d=D),
            in1=inv_bc,
            op=mybir.AluOpType.mult,
        ).then_inc(V, 1)
        nc.vector.wait_ge(V, 3)
        for k in range(NCH):
            f0 = k * CBLK * D
            nc.vector.wait_ge(G[k // CPG], 16)
            nc.vector.tensor_sub(c_t[:, f0:f0 + CBLK * D],
                                 nx[:, f0:f0 + CBLK * D],
                                 c_t[:, f0:f0 + CBLK * D]).then_inc(V, 1)

    nc.compile()
    return nc


def _get_nc():
    if "nc" not in _CACHE:
        _CACHE["nc"] = _build()
    return _CACHE["nc"]


def _prep_labels(lab_shard):
    """int16 idx layout for dma_gather: gather slot i = j*128+p must hold
    the label of x row p*16+j (so dst[i%128, i//128] aligns with x_t);
    then wrap slots into 16 partitions (idxs[c, s] = slot s*16+c) and
    replicate for the 8 Q7 core groups."""
    slots = lab_shard.reshape(P, J).T.reshape(-1)          # slot j*128+p
    wrapped = slots.reshape(ROWS // 16, 16).T              # [16, ROWS/16]
    return np.ascontiguousarray(
        np.tile(wrapped, (8, 1)).astype(np.int16))         # [128, ROWS/16]


def _run(x, labels, centers, trace=False):
    from concourse.bass_utils import run_bass_kernel_spmd

    x = np.ascontiguousarray(np.asarray(x, dtype=np.float32))
    labels = np.asarray(labels).astype(np.int16)
    centers = np.ascontiguousarray(np.asarray(centers, dtype=np.float32))

    in_maps = []
    for i in range(N_CORES):
        in_maps.append({
            "x": x[i * ROWS:(i + 1) * ROWS],
            "labels": _prep_labels(labels[i * ROWS:(i + 1) * ROWS]),
            "centers": centers,
        })
    res = run_bass_kernel_spmd(_get_nc(), in_maps,
                               core_ids=list(range(N_CORES)), trace=trace)
    total = np.float64(0.0)
    for r in res.results:
        total += np.float64(r["out"].sum(dtype=np.float64))
    loss = np.array(np.float32(total / B))
    return loss, res


def kernel(x, labels, centers):
    loss, _ = _run(x, labels, centers, trace=False)
    return loss
